# revision 25
# baseline (speedup 1.0000x reference)
"""Trainium2 Bass kernel for a pre-norm transformer block with dilated
windowed causal attention (B=2, L=2048, D=512, H=8, DIL=2, WIN=256,
HIDDEN=2048).

Sharding: 8 cores = batch(2) x sequence-chunk(4 x 512 tokens). Each core
receives its 512-token chunk plus a 256-token halo (keys/values only) and
computes the full block for its tokens; no collectives.

v3 layout notes:
  Tokens are permuted host-side into parity-stream-major order (all even
  tokens, then all odd) so every dilated-attention access is contiguous;
  the host un-permutes the output.  x ships bf16.  Weight DMAs are
  released by data-dependent gate copies so x owns the HBM line first.
  Attention per head: one [128,1024] PSUM scores tile -> ONE exp -> ONE
  mask multiply -> PV.  Even chains put features in psum rows 0:64 with a
  ones-row denominator in row 64; odd chains put features in rows 64:128
  with an all-ones M=64 matmul replicating the denominator into rows
  0:64.  A single cast evacuates each chain; denominators broadcast via
  1-contraction matmuls; one reciprocal_approx_fast per head pair; the
  normalize multiplies run in bf16 2x mode.  bv is folded into the
  out-proj bias host-side.
"""
import os
import sys

os.environ.setdefault("MYCRO_LOCAL_CACHE", "1")
if "/opt/trn_rl_repo" not in sys.path:
    sys.path.insert(0, "/opt/trn_rl_repo")

import numpy as np

B, L, D, H, HD = 2, 2048, 512, 8, 64
HIDDEN = 4 * D
P = 128
CH = 512            # own tokens per core
HALO = 256
T = CH + HALO       # 768
NCORES = 8
EPS = 1e-5
SL = T // 2         # 384 keys per parity stream
SQ = CH // 2        # 256 queries per parity stream
SW = 128            # causal window in stream coords
SCALE = 1.0 / 8.0   # 1/sqrt(HD)

NT = T // P         # 6
NO = CH // P        # 4
ND = D // P         # 4
NHID = HIDDEN // P  # 16

# FFN weights ship as fp8e4m3 scaled by a fixed power of two (values are
# ~N(0, 0.02^2); 2048x puts absmax around 200 of the 240 fp8 range, and the
# rare >5.9-sigma outlier is clipped host-side).
FFN_WSCALE = 2048.0

# stream-major token order: xc rows = [even tokens (384), odd tokens (384)]
# xT cols likewise; own tokens sit at cols [128:384] and [512:768].
OWN_TILE = [1, 2, 4, 5]   # xc tile holding out-proj token block tt

# scores-tile column offsets: [kt0 s0|s1, kt1 s0|s1, kt2 s0|s1]
def _scol(kt, stp):
    return (stp * 128, 256 + stp * 256, 768 + stp * 128)[kt]

_nc = None
LAST_EXEC_NS = None
LAST_RESULTS = None


def _body(ctx, tc, I, y):
    import concourse.bass as bass  # noqa: F401
    from concourse import mybir
    from concourse.masks import make_identity

    nc = tc.nc
    f32 = mybir.dt.float32
    bf16 = mybir.dt.bfloat16
    fp8 = mybir.dt.float8e4
    AF = mybir.ActivationFunctionType
    OP = mybir.AluOpType
    PM = mybir.MatmulPerfMode

    consts = ctx.enter_context(tc.tile_pool(name="consts", bufs=1))
    big = ctx.enter_context(tc.tile_pool(name="big", bufs=1))
    work = ctx.enter_context(tc.tile_pool(name="work", bufs=4))
    pexp = ctx.enter_context(tc.tile_pool(name="pexp", bufs=3))
    pps = ctx.enter_context(tc.tile_pool(name="pps", bufs=4, space="PSUM"))
    psc = ctx.enter_context(tc.tile_pool(name="psc", bufs=2, space="PSUM"))

    mm = nc.tensor.matmul

    def bcast(ap, p=P):
        return bass.AP(tensor=ap.tensor, offset=ap.offset,
                       ap=[[0, p]] + [list(d) for d in ap.ap])

    # ---------- constants / warmup ----------
    junk_in = consts.tile([P, CH], bf16, tag="junk")
    nc.vector.memset(junk_in, 0.25)
    for _ in range(16):
        jp = pps.tile([P, CH], f32, tag="ps")
        mm(jp, junk_in[:, 0:P], junk_in, start=True, stop=True)

    ident = consts.tile([P, P], bf16, tag="ident")
    make_identity(nc, ident)
    epst = consts.tile([P, 1], f32, tag="eps")
    nc.vector.memset(epst, EPS)
    onesd = consts.tile([P, 64], bf16, tag="onesd")
    nc.vector.memset(onesd, 1.0)

    # ---------- input DMAs (x first; weights gated) ----------
    x_sb = big.tile([P, NT, D], bf16, tag="x")
    nc.sync.dma_start(out=x_sb, in_=I["xc"])
    # packed [bq(4) | bk(4) | b1(16)] per-partition constants: one DMA
    bcons = consts.tile([P, 24], f32, tag="bcons")
    nc.sync.dma_start(out=bcons, in_=I["bcons"])
    bq_sb = bcons[:, 0:4]
    bk_sb = bcons[:, 4:8]
    b1_sb = bcons[:, 8:24]
    masks_sb = consts.tile([P, 1024], bf16, tag="masks")
    bo_sb = consts.tile([P, D], f32, tag="bo")
    b2_sb = consts.tile([P, D], f32, tag="b2")

    wq_sb = big.tile([P, ND, D], bf16, tag="wq")
    wk_sb = big.tile([P, ND, D], bf16, tag="wk")
    wv_sb = big.tile([P, ND, D], bf16, tag="wv")
    wo_sb = big.tile([P, ND, D], bf16, tag="wo")
    w1_sb = big.tile([P, ND, HIDDEN], fp8, tag="w1")
    w2_sb = big.tile([P, NHID, D], fp8, tag="w2")

    # ---------- LN1 (token-major stats; own tiles first) ----------
    xhat = big.tile([P, NT, D], bf16, tag="xhat")
    ln_order = [1, 2, 4, 5, 0, 3]
    for it, j in enumerate(ln_order):
        st = work.tile([P, 6], f32, tag="bnst")
        nc.vector.bn_stats(st, x_sb[:, j, :])
        # gates: release each weight DMA only once LN1 has consumed enough
        # of x, so x owns the HBM line first (real RAW dep, can't hoist).
        if it == 0:
            nc.vector.tensor_copy(wq_sb[0:1, 0:1, 0:1], st[0:1, 0:1])
        elif it == 2:
            nc.vector.tensor_copy(wk_sb[0:1, 0:1, 0:1], st[0:1, 0:1])
            nc.vector.tensor_copy(masks_sb[0:1, 0:1], st[0:1, 0:1])
        elif it == 4:
            nc.vector.tensor_copy(wv_sb[0:1, 0:1, 0:1], st[0:1, 0:1])
            nc.vector.tensor_copy(bo_sb[0:1, 0:1], st[0:1, 0:1])
            nc.vector.tensor_copy(b2_sb[0:1, 0:1], st[0:1, 0:1])
        mv = work.tile([P, 2], f32, tag="bnmv")
        nc.vector.bn_aggr(mv, st)
        r = work.tile([P, 1], f32, tag="lnr")
        nc.scalar.activation(r, mv[:, 1:2], AF.Sqrt, bias=epst, scale=1.0)
        r2 = work.tile([P, 1], f32, tag="lnr2")
        nc.vector.reciprocal(r2, r)
        nc.vector.tensor_scalar(
            out=xhat[:, j, :], in0=x_sb[:, j, :],
            scalar1=mv[:, 0:1], scalar2=r2,
            op0=OP.subtract, op1=OP.mult,
        )

    nc.sync.dma_start(out=wq_sb, in_=I["wqT"])
    nc.sync.dma_start(out=wk_sb, in_=I["wkT"])
    nc.sync.dma_start(out=wv_sb, in_=I["wvT"])
    nc.sync.dma_start(out=masks_sb, in_=I["masks"])
    nc.gpsimd.dma_start(out=bo_sb, in_=bcast(I["bo"]))
    nc.gpsimd.dma_start(out=b2_sb, in_=bcast(I["b2"]))

    # ---------- transpose x_hat -> x_hat^T [d, t] ----------
    xT = big.tile([P, ND, T], bf16, tag="xT")
    xTr = xT.rearrange("p d (s c) -> p d s c", s=2)   # [P, ND, 2, 384]
    for pi, j0 in enumerate((1, 4, 0)):
        j1 = j0 + 1 if j0 != 0 else 3
        for dt_ in range(ND):
            pt = pps.tile([P, 2 * P], bf16, tag="ps")
            for jj, j in enumerate((j0, j1)):
                nc.tensor.transpose(pt[:, jj * P:(jj + 1) * P],
                                    xhat[:, j, dt_ * P:(dt_ + 1) * P], ident)
            if j0 == 0:
                dst = xTr[:, dt_, :, 0:P]           # cols 0:128 and 384:512
            else:
                dst = xT[:, dt_, j0 * P:(j0 + 2) * P]
            if dt_ % 2 == 0:
                nc.vector.tensor_copy(dst, pt)
            else:
                nc.scalar.copy(dst, pt)

    # ---------- QKV ----------
    # Q^T [o, own t] (own tokens = stream cols 128:384 per stream)
    qT = big.tile([P, 4, CH], bf16, tag="qT")
    for ot in range(4):
        ps = pps.tile([P, CH], f32, tag="ps")
        for dt_ in range(ND):
            mm(ps, wq_sb[:, dt_, ot * P:(ot + 1) * P], xTr[:, dt_, :, P:SL],
               start=(dt_ == 0), stop=(dt_ == ND - 1))
        nc.scalar.activation(qT[:, ot, :], ps, AF.Identity,
                             bias=bq_sb[:, ot:ot + 1], scale=1.0)

    # K^T: own tokens first, then halo
    kT = big.tile([P, 4, T], bf16, tag="kT")
    kTr = kT.rearrange("p o (s c) -> p o s c", s=2)
    for ot in range(4):
        ps = pps.tile([P, CH], f32, tag="ps")
        for dt_ in range(ND):
            mm(ps, wk_sb[:, dt_, ot * P:(ot + 1) * P], xTr[:, dt_, :, P:SL],
               start=(dt_ == 0), stop=(dt_ == ND - 1))
        nc.scalar.activation(kTr[:, ot, :, P:SL], ps, AF.Identity,
                             bias=bk_sb[:, ot:ot + 1], scale=1.0)

    # V token-major per parity stream (own blocks first; halo last)
    v_sb = big.tile([P, 6, H, 65], bf16, tag="v")
    for i in range(6):
        nc.vector.memset(v_sb[:, i, :, 64:65], 1.0)

    def emit_v_block(stp, i):
        ps = pps.tile([P, D], f32, tag="ps")
        c0 = SL * stp + P * i
        for dt_ in range(ND):
            mm(ps, xT[:, dt_, c0:c0 + P], wv_sb[:, dt_, :],
               start=(dt_ == 0), stop=(dt_ == ND - 1))
        nc.vector.tensor_copy(
            v_sb[:, stp * 3 + i, :, 0:64],
            ps.rearrange("p (h c) -> p h c", h=H))

    for i in (1, 2):
        for stp in range(2):
            emit_v_block(stp, i)

    for ot in range(4):
        ps = pps.tile([P, 256], f32, tag="ps")
        for dt_ in range(ND):
            mm(ps, wk_sb[:, dt_, ot * P:(ot + 1) * P], xTr[:, dt_, :, 0:P],
               start=(dt_ == 0), stop=(dt_ == ND - 1))
        nc.scalar.activation(kTr[:, ot, :, 0:P], ps, AF.Identity,
                             bias=bk_sb[:, ot:ot + 1], scale=1.0)
        if ot == 1:
            # release late weights once x/wq/wk/wv traffic has drained
            nc.vector.tensor_copy(wo_sb[0:1, 0:1, 0:1], v_sb[0:1, 1, 0, 0:1])
            nc.vector.tensor_copy(w1_sb[0:1, 0:1, 0:1], v_sb[0:1, 1, 0, 0:1])
            nc.vector.tensor_copy(w2_sb[0:1, 0:1, 0:1], v_sb[0:1, 1, 0, 0:1])

    for stp in range(2):
        emit_v_block(stp, 0)

    nc.sync.dma_start(out=wo_sb, in_=I["woT"])
    nc.sync.dma_start(out=w1_sb, in_=I["w1T"])
    nc.sync.dma_start(out=w2_sb, in_=I["w2T"])

    # pre-add the out-proj bias into the residual source during slack time
    for tt in range(NO):
        nc.gpsimd.tensor_add(x_sb[:, OWN_TILE[tt], :],
                             x_sb[:, OWN_TILE[tt], :], bo_sb)

    # ---------- attention ----------
    # oU: staging [128, chain, q]; even chain ci: feat rows 0:64 + den row
    # 64; odd chain: den (replicated) rows 0:64 + feat rows 64:128.
    oU = big.tile([P, 8, CH], bf16, tag="oU")
    oT = big.tile([P, 4, CH], bf16, tag="oT")

    def emit_S(ci, hp, hh):
        lo = hh * 64
        sc = psc.tile([P, 1024], f32, tag="sc")
        for kt in range(3):
            qw = SQ if kt == 1 else P
            for stp in range(2):
                q0 = stp * SQ + (0 if kt < 2 else P)
                mm(sc[:, _scol(kt, stp):_scol(kt, stp) + qw],
                   kT[lo:lo + 64, hp, SL * stp + P * kt:SL * stp + P * kt + P],
                   qT[lo:lo + 64, hp, q0:q0 + qw],
                   start=True, stop=True)
        p_sb = pexp.tile([P, 1024], bf16, tag="p_sb")
        nc.scalar.activation(p_sb, sc, AF.Exp, scale=SCALE)
        if ci % 2 == 0:
            nc.gpsimd.tensor_mul(p_sb, p_sb, masks_sb)
        else:
            nc.vector.tensor_mul(p_sb, p_sb, masks_sb)
        return p_sb

    def emit_PV(ci, hp, hh, p_sb):
        h = 2 * hp + hh
        po = pps.tile([P, CH], f32, tag="ps")
        ncol = 65 if hh == 0 else 64
        base = 0 if hh == 0 else 64
        for stp in range(2):
            qa = stp * SQ
            regions = (
                (qa, (_scol(0, stp), 0), (_scol(1, stp), 1)),
                (qa + P, (_scol(1, stp) + P, 1), (_scol(2, stp), 2)),
            )
            for q_out, (cA, iA), (cB, iB) in regions:
                mm(po[base:base + ncol, q_out:q_out + P],
                   v_sb[:, stp * 3 + iA, h, 0:ncol],
                   p_sb[:, cA:cA + P], start=True, stop=False)
                mm(po[base:base + ncol, q_out:q_out + P],
                   v_sb[:, stp * 3 + iB, h, 0:ncol],
                   p_sb[:, cB:cB + P], start=False, stop=True)
                if hh == 1:  # denominator, replicated into rows 0:64
                    mm(po[0:64, q_out:q_out + P], onesd,
                       p_sb[:, cA:cA + P], start=True, stop=False)
                    mm(po[0:64, q_out:q_out + P], onesd,
                       p_sb[:, cB:cB + P], start=False, stop=True)
        span = 65 if hh == 0 else P
        if ci % 2 == 1:
            nc.scalar.copy(oU[0:span, ci, :], po[0:span, :])
        else:
            nc.vector.tensor_copy(oU[0:span, ci, :], po[0:span, :])

    def emit_norm(hp):
        rb_ps = pps.tile([P, CH], f32, tag="ps")
        mm(rb_ps[0:64, :], onesd[64:65, :], oU[64:65, 2 * hp, :],
           start=True, stop=True)
        mm(rb_ps[64:128, :], onesd[0:1, :], oU[0:1, 2 * hp + 1, :],
           start=True, stop=True)
        rb = work.tile([P, CH], f32, tag="rb")
        nc.vector.reciprocal_approx_fast(out=rb, in_=rb_ps)
        rbh = work.tile([P, CH], bf16, tag="rbh")
        nc.vector.tensor_copy(rbh, rb)
        for hh in range(2):
            lo = hh * 64
            nc.vector.tensor_mul(oT[lo:lo + 64, hp, :],
                                 oU[lo:lo + 64, 2 * hp + hh, :],
                                 rbh[lo:lo + 64, :])

    chains = [(hp, hh) for hp in range(4) for hh in range(2)]
    prev = None
    for ci, (hp, hh) in enumerate(chains):
        p_sb = emit_S(ci, hp, hh)
        if prev is not None:
            emit_PV(*prev)
            if prev[2] == 1:
                emit_norm(prev[1])
        prev = (ci, hp, hh, p_sb)
    emit_PV(*prev)
    emit_norm(prev[1])

    # ---------- out projection + residual ----------
    res1 = big.tile([P, NO, D], f32, tag="res1")
    for tt in range(NO):
        ps = pps.tile([P, D], f32, tag="ps")
        for dt_ in range(ND):
            mm(ps, oT[:, dt_, tt * P:(tt + 1) * P], wo_sb[:, dt_, :],
               start=(dt_ == 0), stop=(dt_ == ND - 1))
        nc.vector.tensor_add(res1[:, tt, :], ps, x_sb[:, OWN_TILE[tt], :])

    # ---------- LN2 ----------
    xhat2 = big.tile([P, NO, D], bf16, tag="xhat2")
    for j in range(NO):
        st = work.tile([P, 6], f32, tag="bnst")
        nc.vector.bn_stats(st, res1[:, j, :])
        mv = work.tile([P, 2], f32, tag="bnmv")
        nc.vector.bn_aggr(mv, st)
        r = work.tile([P, 1], f32, tag="lnr")
        nc.scalar.activation(r, mv[:, 1:2], AF.Sqrt, bias=epst, scale=1.0)
        r2 = work.tile([P, 1], f32, tag="lnr2")
        nc.vector.reciprocal(r2, r)
        nc.vector.tensor_scalar(
            out=xhat2[:, j, :], in0=res1[:, j, :],
            scalar1=mv[:, 0:1], scalar2=r2,
            op0=OP.subtract, op1=OP.mult,
        )

    for tt in range(NO):
        nc.gpsimd.tensor_add(res1[:, tt, :], res1[:, tt, :], b2_sb)

    x2T = big.tile([P, ND, CH], fp8, tag="x2T")
    for dt_ in range(ND):
        for j0 in (0, 2):
            pt = pps.tile([P, 2 * P], bf16, tag="ps")
            for jj in range(2):
                nc.tensor.transpose(pt[:, jj * P:(jj + 1) * P],
                                    xhat2[:, j0 + jj, dt_ * P:(dt_ + 1) * P], ident)
            if dt_ % 2 == 0:
                nc.vector.tensor_copy(x2T[:, dt_, j0 * P:(j0 + 2) * P], pt)
            else:
                nc.scalar.copy(x2T[:, dt_, j0 * P:(j0 + 2) * P], pt)

    # ---------- FFN1 (+gelu), feature-major G^T [h, t]; fp8 DoubleRow ----------
    g_sb = big.tile([P, NHID, CH], fp8, tag="g")
    for ht in range(NHID):
        ps = pps.tile([P, CH], f32, tag="ps")
        for dt_ in (0, 2):
            mm(ps, w1_sb[:, dt_:dt_ + 2, ht * P:(ht + 1) * P],
               x2T[:, dt_:dt_ + 2, :],
               start=(dt_ == 0), stop=(dt_ == 2), perf_mode=PM.DoubleRow)
        nc.scalar.activation(g_sb[:, ht, :], ps, AF.Gelu,
                             bias=b1_sb[:, ht:ht + 1], scale=1.0 / FFN_WSCALE)

    # ---------- FFN2 + residual; fp8 DoubleRow ----------
    fin = big.tile([P, NO, D], f32, tag="fin")
    yr = y.rearrange("(j p) d -> p j d", p=P)
    for tt in range(NO):
        ps = pps.tile([P, D], f32, tag="ps")
        for ht in range(0, NHID, 2):
            mm(ps, g_sb[:, ht:ht + 2, tt * P:(tt + 1) * P],
               w2_sb[:, ht:ht + 2, :],
               start=(ht == 0), stop=(ht == NHID - 2), perf_mode=PM.DoubleRow)
        nc.vector.scalar_tensor_tensor(
            out=fin[:, tt, :], in0=ps, scalar=1.0 / FFN_WSCALE,
            in1=res1[:, tt, :], op0=OP.mult, op1=OP.add)
        nc.sync.dma_start(out=yr[:, tt, :], in_=fin[:, tt, :])


def _build():
    from contextlib import ExitStack

    import concourse.bacc as bacc
    import concourse.tile as tile
    from concourse import mybir

    f32 = mybir.dt.float32
    bf16 = mybir.dt.bfloat16
    fp8 = mybir.dt.float8e4
    nc = bacc.Bacc("TRN2", target_bir_lowering=False, debug=False,
                   enable_asserts=False, num_devices=NCORES)
    I = {}

    def inp(name, shape, dt_):
        I[name] = nc.dram_tensor(name, list(shape), dt_, kind="ExternalInput").ap()

    inp("xc", (P, NT, D), bf16)
    inp("wqT", (P, ND, D), bf16)
    inp("wkT", (P, ND, D), bf16)
    inp("wvT", (P, ND, D), bf16)
    inp("bcons", (P, 24), f32)
    inp("woT", (P, ND, D), bf16)
    inp("bo", (D,), f32)
    inp("w1T", (P, ND, HIDDEN), fp8)
    inp("w2T", (P, NHID, D), fp8)
    inp("b2", (D,), f32)
    inp("masks", (P, 1024), bf16)
    y = nc.dram_tensor("y", [CH, D], f32, kind="ExternalOutput").ap()

    with tile.TileContext(nc) as tc:
        with ExitStack() as ctx:
            _body(ctx, tc, I, y)
    nc.compile()
    return nc


def _host_masks():
    import ml_dtypes
    sk = np.arange(SL)[:, None]
    sq = np.arange(SL - SQ, SL)[None, :]
    valid = ((sq - sk >= 0) & (sq - sk <= SW)).astype(np.float32)  # [384, 256]
    kt0 = valid[0:P, 0:P]           # keys 0:128, queries 0:128
    kt1 = valid[P:2 * P, :]         # keys 128:256, all queries
    kt2 = valid[2 * P:3 * P, P:SQ]  # keys 256:384, queries 128:256
    m = np.concatenate([kt0, kt0, kt1, kt1, kt2, kt2], axis=1)  # [128, 1024]
    m = m.astype(ml_dtypes.bfloat16)
    m0 = m.copy()
    m0[:, 0:256] = 0.0  # first chunk of each batch: halo keys invalid
    return np.ascontiguousarray(m), np.ascontiguousarray(m0)


def get_nc():
    global _nc
    if _nc is None:
        _nc = _build()
    return _nc


def _pmaj(a, p=P):
    """[N*p, F...] row-major -> [p, N, F...] partition-major contiguous."""
    n = a.shape[0] // p
    return np.ascontiguousarray(
        a.reshape((n, p) + a.shape[1:]).transpose((1, 0) + tuple(range(2, a.ndim + 1))))


def make_in_maps(inputs):
    import ml_dtypes
    f = np.float32
    bf = ml_dtypes.bfloat16
    x = np.asarray(inputs["x"], f)
    qkv_w = np.asarray(inputs["qkv_w"], f)
    n1w = np.asarray(inputs["norm1_w"], f)
    n1b = np.asarray(inputs["norm1_b"], f)
    wqkv_f = qkv_w * n1w[None, :]
    bqkv = qkv_w @ n1b + np.asarray(inputs["qkv_b"], f)
    wT = np.ascontiguousarray(wqkv_f.T)        # [D, 3D]
    wqT = _pmaj(wT[:, 0:D].copy().astype(bf))
    wkT = _pmaj(wT[:, D:2 * D].copy().astype(bf))
    wvT = _pmaj(wT[:, 2 * D:3 * D].copy().astype(bf))
    bq = np.ascontiguousarray(bqkv[0:D].reshape(4, P).T)
    bk = np.ascontiguousarray(bqkv[D:2 * D].reshape(4, P).T)
    bv = bqkv[2 * D:3 * D]

    out_w = np.asarray(inputs["out_w"], f)
    woT = _pmaj(np.ascontiguousarray(out_w.T).astype(bf))
    # fold the V bias through the out projection (softmax weights sum to 1)
    bo = np.ascontiguousarray(np.asarray(inputs["out_b"], f) + out_w @ bv)

    fp8 = ml_dtypes.float8_e4m3
    w1 = np.asarray(inputs["ffn_w1"], f)
    n2w = np.asarray(inputs["norm2_w"], f)
    n2b = np.asarray(inputs["norm2_b"], f)
    w1T = _pmaj(np.clip(np.ascontiguousarray((w1 * n2w[None, :]).T)
                        * FFN_WSCALE, -240, 240).astype(fp8))
    b1v = w1 @ n2b + np.asarray(inputs["ffn_b1"], f)
    b1 = np.ascontiguousarray(b1v.reshape(NHID, P).T)
    w2T = _pmaj(np.clip(np.ascontiguousarray(np.asarray(inputs["ffn_w2"], f).T)
                        * FFN_WSCALE, -240, 240).astype(fp8))
    b2 = np.ascontiguousarray(np.asarray(inputs["ffn_b2"], f))

    bcons = np.ascontiguousarray(np.concatenate([bq, bk, b1], axis=1))
    masks, masks0 = _host_masks()
    shared = dict(wqT=wqT, wkT=wkT, wvT=wvT, bcons=bcons, woT=woT, bo=bo,
                  w1T=w1T, w2T=w2T, b2=b2)
    # stream-major permutation of the 768 halo+own tokens
    perm = np.concatenate([np.arange(0, T, 2), np.arange(1, T, 2)])
    in_maps = []
    for c in range(NCORES):
        b_, i = divmod(c, 4)
        own = x[b_, i * CH:(i + 1) * CH]
        if i == 0:
            halo = np.zeros((HALO, D), f)
        else:
            halo = x[b_, i * CH - HALO:i * CH]
        xc = np.concatenate([halo, own], 0)[perm]
        xc = _pmaj(xc.astype(bf))
        in_maps.append(dict(xc=xc, masks=(masks if i > 0 else masks0), **shared))
    return in_maps


def kernel(**inputs):
    global LAST_EXEC_NS, LAST_RESULTS
    from concourse.bass_utils import run_bass_kernel_spmd

    nc = get_nc()
    in_maps = make_in_maps(inputs)
    trace = bool(int(os.environ.get("BASS_KERNEL_TRACE", "0")))
    res = run_bass_kernel_spmd(nc, in_maps, core_ids=list(range(NCORES)),
                               trace=trace)
    LAST_EXEC_NS = res.exec_time_ns
    LAST_RESULTS = res
    out = np.zeros((B, L, D), np.float32)
    # kernel y rows are stream-major own tokens: un-permute
    operm = np.concatenate([np.arange(0, CH, 2), np.arange(1, CH, 2)])
    for c, r in enumerate(res.results):
        b_, i = divmod(c, 4)
        out[b_, i * CH + operm] = r["y"]
    return out


# revision 30
# speedup vs baseline: 1.1793x; 1.1793x over previous
"""Trainium2 Bass kernel for a pre-norm transformer block with dilated
windowed causal attention (B=2, L=2048, D=512, H=8, DIL=2, WIN=256,
HIDDEN=2048).

Sharding: 8 cores = batch(2) x sequence-chunk(4 x 512 tokens). Each core
receives its 512-token chunk plus a 256-token halo (keys/values only) and
computes the full block for its tokens; no collectives.

v3 layout notes:
  Tokens are permuted host-side into parity-stream-major order (all even
  tokens, then all odd) so every dilated-attention access is contiguous;
  the host un-permutes the output.  x ships bf16.  Weight DMAs are
  released by data-dependent gate copies so x owns the HBM line first.
  Attention per head: one [128,1024] PSUM scores tile -> ONE exp -> ONE
  mask multiply -> PV.  Even chains put features in psum rows 0:64 with a
  ones-row denominator in row 64; odd chains put features in rows 64:128
  with an all-ones M=64 matmul replicating the denominator into rows
  0:64.  A single cast evacuates each chain; denominators broadcast via
  1-contraction matmuls; one reciprocal_approx_fast per head pair; the
  normalize multiplies run in bf16 2x mode.  bv is folded into the
  out-proj bias host-side.
"""
import os
import sys

os.environ.setdefault("MYCRO_LOCAL_CACHE", "1")
if "/opt/trn_rl_repo" not in sys.path:
    sys.path.insert(0, "/opt/trn_rl_repo")

import numpy as np

B, L, D, H, HD = 2, 2048, 512, 8, 64
HIDDEN = 4 * D
P = 128
CH = 512            # own tokens per core
HALO = 256
T = CH + HALO       # 768
NCORES = 8
EPS = 1e-5
SL = T // 2         # 384 keys per parity stream
SQ = CH // 2        # 256 queries per parity stream
SW = 128            # causal window in stream coords
SCALE = 1.0 / 8.0   # 1/sqrt(HD)

NT = T // P         # 6
NO = CH // P        # 4
ND = D // P         # 4
NHID = HIDDEN // P  # 16

# FFN weights ship as fp8e4m3 scaled by a fixed power of two (values are
# ~N(0, 0.02^2); 2048x puts absmax around 200 of the 240 fp8 range, and the
# rare >5.9-sigma outlier is clipped host-side).
FFN_WSCALE = 2048.0

# stream-major token order: xc rows = [even tokens (384), odd tokens (384)]
# xT cols likewise; own tokens sit at cols [128:384] and [512:768].
OWN_TILE = [1, 2, 4, 5]   # xc tile holding out-proj token block tt

# scores-tile column offsets: [kt0 s0|s1, kt1 s0|s1, kt2 s0|s1]
def _scol(kt, stp):
    return (stp * 128, 256 + stp * 256, 768 + stp * 128)[kt]

_nc = None
LAST_EXEC_NS = None
LAST_RESULTS = None


def _body(ctx, tc, I, y):
    import concourse.bass as bass  # noqa: F401
    from concourse import mybir
    from concourse.masks import make_identity

    nc = tc.nc
    f32 = mybir.dt.float32
    bf16 = mybir.dt.bfloat16
    fp8 = mybir.dt.float8e4
    AF = mybir.ActivationFunctionType
    OP = mybir.AluOpType
    PM = mybir.MatmulPerfMode

    consts = ctx.enter_context(tc.tile_pool(name="consts", bufs=1))
    big = ctx.enter_context(tc.tile_pool(name="big", bufs=1))
    work = ctx.enter_context(tc.tile_pool(name="work", bufs=4))
    pexp = ctx.enter_context(tc.tile_pool(name="pexp", bufs=3))
    pps = ctx.enter_context(tc.tile_pool(name="pps", bufs=4, space="PSUM"))
    psc = ctx.enter_context(tc.tile_pool(name="psc", bufs=2, space="PSUM"))

    mm = nc.tensor.matmul

    def bcast(ap, p=P):
        return bass.AP(tensor=ap.tensor, offset=ap.offset,
                       ap=[[0, p]] + [list(d) for d in ap.ap])

    # ---------- constants / warmup ----------
    junk_in = consts.tile([P, CH], bf16, tag="junk")
    nc.vector.memset(junk_in, 0.25)
    for _ in range(16):
        jp = pps.tile([P, CH], f32, tag="ps")
        mm(jp, junk_in[:, 0:P], junk_in, start=True, stop=True)

    ident = consts.tile([P, P], bf16, tag="ident")
    make_identity(nc, ident)
    epst = consts.tile([P, 1], f32, tag="eps")
    nc.vector.memset(epst, EPS)
    onesd = consts.tile([P, 64], bf16, tag="onesd")
    nc.vector.memset(onesd, 1.0)

    # ---------- input DMAs (x first; weights gated) ----------
    x_sb = big.tile([P, NT, D], bf16, tag="x")
    nc.sync.dma_start(out=x_sb, in_=I["xc"])
    # packed [bq(4) | bk(4) | b1(16)] per-partition constants: one DMA
    bcons = consts.tile([P, 24], f32, tag="bcons")
    nc.sync.dma_start(out=bcons, in_=I["bcons"])
    bq_sb = bcons[:, 0:4]
    bk_sb = bcons[:, 4:8]
    b1_sb = bcons[:, 8:24]
    masks_sb = consts.tile([P, 1024], bf16, tag="masks")
    bo_sb = consts.tile([P, D], f32, tag="bo")
    b2_sb = consts.tile([P, D], f32, tag="b2")

    wq_sb = big.tile([P, ND, D], bf16, tag="wq")
    wk_sb = big.tile([P, ND, D], bf16, tag="wk")
    wv_sb = big.tile([P, ND, D], bf16, tag="wv")
    wo_sb = big.tile([P, ND, D], bf16, tag="wo")
    w1_sb = big.tile([P, ND, HIDDEN], fp8, tag="w1")
    w2_sb = big.tile([P, NHID, D], fp8, tag="w2")

    # ---------- LN1 (token-major stats; own tiles first) ----------
    xhat = big.tile([P, NT, D], bf16, tag="xhat")
    ln_order = [1, 2, 4, 5, 0, 3]
    for it, j in enumerate(ln_order):
        st = work.tile([P, 6], f32, tag="bnst")
        nc.vector.bn_stats(st, x_sb[:, j, :])
        # gates: release each weight DMA only once LN1 has consumed enough
        # of x, so x owns the HBM line first (real RAW dep, can't hoist).
        if it == 0:
            nc.vector.tensor_copy(wq_sb[0:1, 0:1, 0:1], st[0:1, 0:1])
        elif it == 2:
            nc.vector.tensor_copy(wk_sb[0:1, 0:1, 0:1], st[0:1, 0:1])
            nc.vector.tensor_copy(masks_sb[0:1, 0:1], st[0:1, 0:1])
        elif it == 4:
            nc.vector.tensor_copy(wv_sb[0:1, 0:1, 0:1], st[0:1, 0:1])
            nc.vector.tensor_copy(bo_sb[0:1, 0:1], st[0:1, 0:1])
            nc.vector.tensor_copy(b2_sb[0:1, 0:1], st[0:1, 0:1])
        mv = work.tile([P, 2], f32, tag="bnmv")
        nc.vector.bn_aggr(mv, st)
        r = work.tile([P, 1], f32, tag="lnr")
        nc.scalar.activation(r, mv[:, 1:2], AF.Sqrt, bias=epst, scale=1.0)
        r2 = work.tile([P, 1], f32, tag="lnr2")
        nc.vector.reciprocal(r2, r)
        nc.vector.tensor_scalar(
            out=xhat[:, j, :], in0=x_sb[:, j, :],
            scalar1=mv[:, 0:1], scalar2=r2,
            op0=OP.subtract, op1=OP.mult,
        )

    nc.sync.dma_start(out=wq_sb, in_=I["wqT"])
    nc.sync.dma_start(out=wk_sb, in_=I["wkT"])
    nc.sync.dma_start(out=wv_sb, in_=I["wvT"])
    nc.sync.dma_start(out=masks_sb, in_=I["masks"])
    nc.gpsimd.dma_start(out=bo_sb, in_=bcast(I["bo"]))
    nc.gpsimd.dma_start(out=b2_sb, in_=bcast(I["b2"]))

    # ---------- transpose x_hat -> x_hat^T [d, t] ----------
    xT = big.tile([P, ND, T], bf16, tag="xT")
    xTr = xT.rearrange("p d (s c) -> p d s c", s=2)   # [P, ND, 2, 384]
    for pi, j0 in enumerate((1, 4, 0)):
        j1 = j0 + 1 if j0 != 0 else 3
        for dt_ in range(ND):
            pt = pps.tile([P, 2 * P], bf16, tag="ps")
            for jj, j in enumerate((j0, j1)):
                nc.tensor.transpose(pt[:, jj * P:(jj + 1) * P],
                                    xhat[:, j, dt_ * P:(dt_ + 1) * P], ident)
            if j0 == 0:
                dst = xTr[:, dt_, :, 0:P]           # cols 0:128 and 384:512
            else:
                dst = xT[:, dt_, j0 * P:(j0 + 2) * P]
            if dt_ % 2 == 0:
                nc.vector.tensor_copy(dst, pt)
            else:
                nc.scalar.copy(dst, pt)

    # ---------- QKV ----------
    # Q^T [o, own t] (own tokens = stream cols 128:384 per stream)
    qT = big.tile([P, 4, CH], bf16, tag="qT")
    for ot in range(4):
        ps = pps.tile([P, CH], f32, tag="ps")
        for dt_ in range(ND):
            mm(ps, wq_sb[:, dt_, ot * P:(ot + 1) * P], xTr[:, dt_, :, P:SL],
               start=(dt_ == 0), stop=(dt_ == ND - 1))
        nc.scalar.activation(qT[:, ot, :], ps, AF.Identity,
                             bias=bq_sb[:, ot:ot + 1], scale=1.0)

    # K^T: own tokens first, then halo
    kT = big.tile([P, 4, T], bf16, tag="kT")
    kTr = kT.rearrange("p o (s c) -> p o s c", s=2)
    for ot in range(4):
        ps = pps.tile([P, CH], f32, tag="ps")
        for dt_ in range(ND):
            mm(ps, wk_sb[:, dt_, ot * P:(ot + 1) * P], xTr[:, dt_, :, P:SL],
               start=(dt_ == 0), stop=(dt_ == ND - 1))
        nc.scalar.activation(kTr[:, ot, :, P:SL], ps, AF.Identity,
                             bias=bk_sb[:, ot:ot + 1], scale=1.0)

    # V token-major per parity stream (own blocks first; halo last)
    v_sb = big.tile([P, 6, H, 65], bf16, tag="v")
    for i in range(6):
        nc.vector.memset(v_sb[:, i, :, 64:65], 1.0)

    def emit_v_block(stp, i):
        ps = pps.tile([P, D], f32, tag="ps")
        c0 = SL * stp + P * i
        for dt_ in range(ND):
            mm(ps, xT[:, dt_, c0:c0 + P], wv_sb[:, dt_, :],
               start=(dt_ == 0), stop=(dt_ == ND - 1))
        nc.vector.tensor_copy(
            v_sb[:, stp * 3 + i, :, 0:64],
            ps.rearrange("p (h c) -> p h c", h=H))

    for i in (1, 2):
        for stp in range(2):
            emit_v_block(stp, i)

    for ot in range(4):
        ps = pps.tile([P, 256], f32, tag="ps")
        for dt_ in range(ND):
            mm(ps, wk_sb[:, dt_, ot * P:(ot + 1) * P], xTr[:, dt_, :, 0:P],
               start=(dt_ == 0), stop=(dt_ == ND - 1))
        nc.scalar.activation(kTr[:, ot, :, 0:P], ps, AF.Identity,
                             bias=bk_sb[:, ot:ot + 1], scale=1.0)
        if ot == 1:
            # release late weights once x/wq/wk/wv traffic has drained
            nc.vector.tensor_copy(wo_sb[0:1, 0:1, 0:1], v_sb[0:1, 1, 0, 0:1])
            nc.vector.tensor_copy(w1_sb[0:1, 0:1, 0:1], v_sb[0:1, 1, 0, 0:1])
            nc.vector.tensor_copy(w2_sb[0:1, 0:1, 0:1], v_sb[0:1, 1, 0, 0:1])

    for stp in range(2):
        emit_v_block(stp, 0)

    nc.sync.dma_start(out=wo_sb, in_=I["woT"])
    nc.sync.dma_start(out=w1_sb, in_=I["w1T"])
    nc.sync.dma_start(out=w2_sb, in_=I["w2T"])

    # pre-add the out-proj bias into the residual source during slack time
    for tt in range(NO):
        nc.gpsimd.tensor_add(x_sb[:, OWN_TILE[tt], :],
                             x_sb[:, OWN_TILE[tt], :], bo_sb)

    # ---------- attention ----------
    # oU: staging [128, chain, q]; even chain ci: feat rows 0:64 + den row
    # 64; odd chain: den (replicated) rows 0:64 + feat rows 64:128.
    oU = big.tile([P, 8, CH], bf16, tag="oU")
    oT = big.tile([P, 4, CH], bf16, tag="oT")

    def emit_S(ci, hp, hh):
        lo = hh * 64
        sc = psc.tile([P, 1024], f32, tag="sc")
        for kt in range(3):
            qw = SQ if kt == 1 else P
            for stp in range(2):
                q0 = stp * SQ + (0 if kt < 2 else P)
                mm(sc[:, _scol(kt, stp):_scol(kt, stp) + qw],
                   kT[lo:lo + 64, hp, SL * stp + P * kt:SL * stp + P * kt + P],
                   qT[lo:lo + 64, hp, q0:q0 + qw],
                   start=True, stop=True)
        p_sb = pexp.tile([P, 1024], bf16, tag="p_sb")
        nc.scalar.activation(p_sb, sc, AF.Exp, scale=SCALE)
        nc.vector.tensor_mul(p_sb, p_sb, masks_sb)
        return p_sb

    def emit_PV(ci, hp, hh, p_sb):
        h = 2 * hp + hh
        po = pps.tile([P, CH], f32, tag="ps")
        ncol = 65 if hh == 0 else 64
        base = 0 if hh == 0 else 64
        for stp in range(2):
            qa = stp * SQ
            regions = (
                (qa, (_scol(0, stp), 0), (_scol(1, stp), 1)),
                (qa + P, (_scol(1, stp) + P, 1), (_scol(2, stp), 2)),
            )
            for q_out, (cA, iA), (cB, iB) in regions:
                mm(po[base:base + ncol, q_out:q_out + P],
                   v_sb[:, stp * 3 + iA, h, 0:ncol],
                   p_sb[:, cA:cA + P], start=True, stop=False)
                mm(po[base:base + ncol, q_out:q_out + P],
                   v_sb[:, stp * 3 + iB, h, 0:ncol],
                   p_sb[:, cB:cB + P], start=False, stop=True)
                if hh == 1:  # denominator, replicated into rows 0:64
                    mm(po[0:64, q_out:q_out + P], onesd,
                       p_sb[:, cA:cA + P], start=True, stop=False)
                    mm(po[0:64, q_out:q_out + P], onesd,
                       p_sb[:, cB:cB + P], start=False, stop=True)
        span = 65 if hh == 0 else P
        if ci % 2 == 1:
            nc.scalar.copy(oU[0:span, ci, :], po[0:span, :])
        else:
            nc.vector.tensor_copy(oU[0:span, ci, :], po[0:span, :])

    def emit_norm(hp):
        rb_ps = pps.tile([P, CH], f32, tag="ps")
        mm(rb_ps[0:64, :], onesd[64:65, :], oU[64:65, 2 * hp, :],
           start=True, stop=True)
        mm(rb_ps[64:128, :], onesd[0:1, :], oU[0:1, 2 * hp + 1, :],
           start=True, stop=True)
        rb = work.tile([P, CH], f32, tag="rb")
        nc.vector.reciprocal_approx_fast(out=rb, in_=rb_ps)
        rbh = work.tile([P, CH], bf16, tag="rbh")
        nc.vector.tensor_copy(rbh, rb)
        for hh in range(2):
            lo = hh * 64
            nc.vector.tensor_mul(oT[lo:lo + 64, hp, :],
                                 oU[lo:lo + 64, 2 * hp + hh, :],
                                 rbh[lo:lo + 64, :])

    chains = [(hp, hh) for hp in range(4) for hh in range(2)]
    prev = None
    for ci, (hp, hh) in enumerate(chains):
        p_sb = emit_S(ci, hp, hh)
        if prev is not None:
            emit_PV(*prev)
            if prev[2] == 1:
                emit_norm(prev[1])
        prev = (ci, hp, hh, p_sb)
    emit_PV(*prev)
    emit_norm(prev[1])

    # ---------- out projection + residual ----------
    res1 = big.tile([P, NO, D], f32, tag="res1")
    for tt in range(NO):
        ps = pps.tile([P, D], f32, tag="ps")
        for dt_ in range(ND):
            mm(ps, oT[:, dt_, tt * P:(tt + 1) * P], wo_sb[:, dt_, :],
               start=(dt_ == 0), stop=(dt_ == ND - 1))
        nc.vector.tensor_add(res1[:, tt, :], ps, x_sb[:, OWN_TILE[tt], :])

    # ---------- LN2 ----------
    xhat2 = big.tile([P, NO, D], bf16, tag="xhat2")
    for j in range(NO):
        st = work.tile([P, 6], f32, tag="bnst")
        nc.vector.bn_stats(st, res1[:, j, :])
        mv = work.tile([P, 2], f32, tag="bnmv")
        nc.vector.bn_aggr(mv, st)
        r = work.tile([P, 1], f32, tag="lnr")
        nc.scalar.activation(r, mv[:, 1:2], AF.Sqrt, bias=epst, scale=1.0)
        r2 = work.tile([P, 1], f32, tag="lnr2")
        nc.vector.reciprocal(r2, r)
        nc.vector.tensor_scalar(
            out=xhat2[:, j, :], in0=res1[:, j, :],
            scalar1=mv[:, 0:1], scalar2=r2,
            op0=OP.subtract, op1=OP.mult,
        )

    for tt in range(NO):
        nc.gpsimd.tensor_add(res1[:, tt, :], res1[:, tt, :], b2_sb)

    x2T = big.tile([P, ND, CH], fp8, tag="x2T")
    for dt_ in range(ND):
        for j0 in (0, 2):
            pt = pps.tile([P, 2 * P], bf16, tag="ps")
            for jj in range(2):
                nc.tensor.transpose(pt[:, jj * P:(jj + 1) * P],
                                    xhat2[:, j0 + jj, dt_ * P:(dt_ + 1) * P], ident)
            if dt_ % 2 == 0:
                nc.vector.tensor_copy(x2T[:, dt_, j0 * P:(j0 + 2) * P], pt)
            else:
                nc.scalar.copy(x2T[:, dt_, j0 * P:(j0 + 2) * P], pt)

    # ---------- FFN, interleaved per ht-pair; fp8 DoubleRow ----------
    # FFN2 accumulates into 4 persistent psums (2 from the now-idle scores
    # pool, 2 from pps) while FFN1+gelu stream the hidden pairs through.
    g_sb = big.tile([P, NHID, CH], fp8, tag="g")
    fp0 = psc.tile([P, 1024], f32, tag="sc")
    fp1 = psc.tile([P, 1024], f32, tag="sc")
    fp2 = pps.tile([P, D], f32, tag="ps")
    fp3 = pps.tile([P, D], f32, tag="ps")
    fview = [fp0[:, 0:D], fp1[:, 0:D], fp2, fp3]
    for pr in range(NHID // 2):
        for ht in (2 * pr, 2 * pr + 1):
            ps = pps.tile([P, CH], f32, tag="ps")
            for dt_ in (0, 2):
                mm(ps, w1_sb[:, dt_:dt_ + 2, ht * P:(ht + 1) * P],
                   x2T[:, dt_:dt_ + 2, :],
                   start=(dt_ == 0), stop=(dt_ == 2), perf_mode=PM.DoubleRow)
            nc.scalar.activation(g_sb[:, ht, :], ps, AF.Gelu,
                                 bias=b1_sb[:, ht:ht + 1],
                                 scale=1.0 / FFN_WSCALE)
        for tt in range(NO):
            mm(fview[tt], g_sb[:, 2 * pr:2 * pr + 2, tt * P:(tt + 1) * P],
               w2_sb[:, 2 * pr:2 * pr + 2, :],
               start=(pr == 0), stop=(pr == NHID // 2 - 1),
               perf_mode=PM.DoubleRow)

    fin = big.tile([P, NO, D], bf16, tag="fin")
    yr = y.rearrange("(j p) d -> p j d", p=P)
    for tt in range(NO):
        nc.vector.scalar_tensor_tensor(
            out=fin[:, tt, :], in0=fview[tt], scalar=1.0 / FFN_WSCALE,
            in1=res1[:, tt, :], op0=OP.mult, op1=OP.add)
        nc.sync.dma_start(out=yr[:, tt, :], in_=fin[:, tt, :])


def _build():
    from contextlib import ExitStack

    import concourse.bacc as bacc
    import concourse.tile as tile
    from concourse import mybir

    f32 = mybir.dt.float32
    bf16 = mybir.dt.bfloat16
    fp8 = mybir.dt.float8e4
    nc = bacc.Bacc("TRN2", target_bir_lowering=False, debug=False,
                   enable_asserts=False, num_devices=NCORES)
    I = {}

    def inp(name, shape, dt_):
        I[name] = nc.dram_tensor(name, list(shape), dt_, kind="ExternalInput").ap()

    inp("xc", (P, NT, D), bf16)
    inp("wqT", (P, ND, D), bf16)
    inp("wkT", (P, ND, D), bf16)
    inp("wvT", (P, ND, D), bf16)
    inp("bcons", (P, 24), f32)
    inp("woT", (P, ND, D), bf16)
    inp("bo", (D,), f32)
    inp("w1T", (P, ND, HIDDEN), fp8)
    inp("w2T", (P, NHID, D), fp8)
    inp("b2", (D,), f32)
    inp("masks", (P, 1024), bf16)
    y = nc.dram_tensor("y", [CH, D], bf16, kind="ExternalOutput").ap()

    with tile.TileContext(nc) as tc:
        with ExitStack() as ctx:
            _body(ctx, tc, I, y)
    nc.compile()
    return nc


def _host_masks():
    import ml_dtypes
    sk = np.arange(SL)[:, None]
    sq = np.arange(SL - SQ, SL)[None, :]
    valid = ((sq - sk >= 0) & (sq - sk <= SW)).astype(np.float32)  # [384, 256]
    kt0 = valid[0:P, 0:P]           # keys 0:128, queries 0:128
    kt1 = valid[P:2 * P, :]         # keys 128:256, all queries
    kt2 = valid[2 * P:3 * P, P:SQ]  # keys 256:384, queries 128:256
    m = np.concatenate([kt0, kt0, kt1, kt1, kt2, kt2], axis=1)  # [128, 1024]
    m = m.astype(ml_dtypes.bfloat16)
    m0 = m.copy()
    m0[:, 0:256] = 0.0  # first chunk of each batch: halo keys invalid
    return np.ascontiguousarray(m), np.ascontiguousarray(m0)


def get_nc():
    global _nc
    if _nc is None:
        _nc = _build()
    return _nc


def _pmaj(a, p=P):
    """[N*p, F...] row-major -> [p, N, F...] partition-major contiguous."""
    n = a.shape[0] // p
    return np.ascontiguousarray(
        a.reshape((n, p) + a.shape[1:]).transpose((1, 0) + tuple(range(2, a.ndim + 1))))


def make_in_maps(inputs):
    import ml_dtypes
    f = np.float32
    bf = ml_dtypes.bfloat16
    x = np.asarray(inputs["x"], f)
    qkv_w = np.asarray(inputs["qkv_w"], f)
    n1w = np.asarray(inputs["norm1_w"], f)
    n1b = np.asarray(inputs["norm1_b"], f)
    wqkv_f = qkv_w * n1w[None, :]
    bqkv = qkv_w @ n1b + np.asarray(inputs["qkv_b"], f)
    wT = np.ascontiguousarray(wqkv_f.T)        # [D, 3D]
    wqT = _pmaj(wT[:, 0:D].copy().astype(bf))
    wkT = _pmaj(wT[:, D:2 * D].copy().astype(bf))
    wvT = _pmaj(wT[:, 2 * D:3 * D].copy().astype(bf))
    bq = np.ascontiguousarray(bqkv[0:D].reshape(4, P).T)
    bk = np.ascontiguousarray(bqkv[D:2 * D].reshape(4, P).T)
    bv = bqkv[2 * D:3 * D]

    out_w = np.asarray(inputs["out_w"], f)
    woT = _pmaj(np.ascontiguousarray(out_w.T).astype(bf))
    # fold the V bias through the out projection (softmax weights sum to 1)
    bo = np.ascontiguousarray(np.asarray(inputs["out_b"], f) + out_w @ bv)

    fp8 = ml_dtypes.float8_e4m3
    w1 = np.asarray(inputs["ffn_w1"], f)
    n2w = np.asarray(inputs["norm2_w"], f)
    n2b = np.asarray(inputs["norm2_b"], f)
    w1T = _pmaj(np.clip(np.ascontiguousarray((w1 * n2w[None, :]).T)
                        * FFN_WSCALE, -240, 240).astype(fp8))
    b1v = w1 @ n2b + np.asarray(inputs["ffn_b1"], f)
    b1 = np.ascontiguousarray(b1v.reshape(NHID, P).T)
    w2T = _pmaj(np.clip(np.ascontiguousarray(np.asarray(inputs["ffn_w2"], f).T)
                        * FFN_WSCALE, -240, 240).astype(fp8))
    b2 = np.ascontiguousarray(np.asarray(inputs["ffn_b2"], f))

    bcons = np.ascontiguousarray(np.concatenate([bq, bk, b1], axis=1))
    masks, masks0 = _host_masks()
    shared = dict(wqT=wqT, wkT=wkT, wvT=wvT, bcons=bcons, woT=woT, bo=bo,
                  w1T=w1T, w2T=w2T, b2=b2)
    # stream-major permutation of the 768 halo+own tokens
    perm = np.concatenate([np.arange(0, T, 2), np.arange(1, T, 2)])
    in_maps = []
    for c in range(NCORES):
        b_, i = divmod(c, 4)
        own = x[b_, i * CH:(i + 1) * CH]
        if i == 0:
            halo = np.zeros((HALO, D), f)
        else:
            halo = x[b_, i * CH - HALO:i * CH]
        xc = np.concatenate([halo, own], 0)[perm]
        xc = _pmaj(xc.astype(bf))
        in_maps.append(dict(xc=xc, masks=(masks if i > 0 else masks0), **shared))
    return in_maps


def kernel(**inputs):
    global LAST_EXEC_NS, LAST_RESULTS
    from concourse.bass_utils import run_bass_kernel_spmd

    nc = get_nc()
    in_maps = make_in_maps(inputs)
    trace = bool(int(os.environ.get("BASS_KERNEL_TRACE", "0")))
    res = run_bass_kernel_spmd(nc, in_maps, core_ids=list(range(NCORES)),
                               trace=trace)
    LAST_EXEC_NS = res.exec_time_ns
    LAST_RESULTS = res
    out = np.zeros((B, L, D), np.float32)
    # kernel y rows are stream-major own tokens: un-permute
    operm = np.concatenate([np.arange(0, CH, 2), np.arange(1, CH, 2)])
    for c, r in enumerate(res.results):
        b_, i = divmod(c, 4)
        out[b_, i * CH + operm] = np.asarray(r["y"], np.float32)
    return out


# revision 32
# speedup vs baseline: 1.2364x; 1.0484x over previous
"""Trainium2 Bass kernel for a pre-norm transformer block with dilated
windowed causal attention (B=2, L=2048, D=512, H=8, DIL=2, WIN=256,
HIDDEN=2048).

Sharding: 8 cores = batch(2) x sequence-chunk(4 x 512 tokens). Each core
receives its 512-token chunk plus a 256-token halo (keys/values only) and
computes the full block for its tokens; no collectives.

v3 layout notes:
  Tokens are permuted host-side into parity-stream-major order (all even
  tokens, then all odd) so every dilated-attention access is contiguous;
  the host un-permutes the output.  x ships bf16.  Weight DMAs are
  released by data-dependent gate copies so x owns the HBM line first.
  Attention per head: one [128,1024] PSUM scores tile -> ONE exp -> ONE
  mask multiply -> PV.  Even chains put features in psum rows 0:64 with a
  ones-row denominator in row 64; odd chains put features in rows 64:128
  with an all-ones M=64 matmul replicating the denominator into rows
  0:64.  A single cast evacuates each chain; denominators broadcast via
  1-contraction matmuls; one reciprocal_approx_fast per head pair; the
  normalize multiplies run in bf16 2x mode.  bv is folded into the
  out-proj bias host-side.
"""
import os
import sys

os.environ.setdefault("MYCRO_LOCAL_CACHE", "1")
if "/opt/trn_rl_repo" not in sys.path:
    sys.path.insert(0, "/opt/trn_rl_repo")

import numpy as np

B, L, D, H, HD = 2, 2048, 512, 8, 64
HIDDEN = 4 * D
P = 128
CH = 512            # own tokens per core
HALO = 256
T = CH + HALO       # 768
NCORES = 8
EPS = 1e-5
SL = T // 2         # 384 keys per parity stream
SQ = CH // 2        # 256 queries per parity stream
SW = 128            # causal window in stream coords
SCALE = 1.0 / 8.0   # 1/sqrt(HD)

NT = T // P         # 6
NO = CH // P        # 4
ND = D // P         # 4
NHID = HIDDEN // P  # 16

# FFN weights ship as fp8e4m3 scaled by a fixed power of two (values are
# ~N(0, 0.02^2); 2048x puts absmax around 200 of the 240 fp8 range, and the
# rare >5.9-sigma outlier is clipped host-side).
FFN_WSCALE = 2048.0

# stream-major token order: xc rows = [even tokens (384), odd tokens (384)]
# xT cols likewise; own tokens sit at cols [128:384] and [512:768].
OWN_TILE = [1, 2, 4, 5]   # xc tile holding out-proj token block tt

# scores-tile column offsets: [kt0 s0|s1, kt1 s0|s1, kt2 s0|s1]
def _scol(kt, stp):
    return (stp * 128, 256 + stp * 256, 768 + stp * 128)[kt]

_nc = None
LAST_EXEC_NS = None
LAST_RESULTS = None


def _body(ctx, tc, I, y):
    import concourse.bass as bass  # noqa: F401
    from concourse import mybir
    from concourse.masks import make_identity

    nc = tc.nc
    f32 = mybir.dt.float32
    bf16 = mybir.dt.bfloat16
    fp8 = mybir.dt.float8e4
    AF = mybir.ActivationFunctionType
    OP = mybir.AluOpType
    PM = mybir.MatmulPerfMode

    consts = ctx.enter_context(tc.tile_pool(name="consts", bufs=1))
    big = ctx.enter_context(tc.tile_pool(name="big", bufs=1))
    work = ctx.enter_context(tc.tile_pool(name="work", bufs=4))
    pexp = ctx.enter_context(tc.tile_pool(name="pexp", bufs=3))
    pps = ctx.enter_context(tc.tile_pool(name="pps", bufs=4, space="PSUM"))
    psc = ctx.enter_context(tc.tile_pool(name="psc", bufs=2, space="PSUM"))

    mm = nc.tensor.matmul

    def bcast(ap, p=P):
        return bass.AP(tensor=ap.tensor, offset=ap.offset,
                       ap=[[0, p]] + [list(d) for d in ap.ap])

    # ---------- constants / warmup ----------
    junk_in = consts.tile([P, CH], bf16, tag="junk")
    nc.vector.memset(junk_in, 0.25)
    for _ in range(16):
        jp = pps.tile([P, CH], f32, tag="ps")
        mm(jp, junk_in[:, 0:P], junk_in, start=True, stop=True)

    ident = consts.tile([P, P], bf16, tag="ident")
    make_identity(nc, ident)
    epst = consts.tile([P, 1], f32, tag="eps")
    nc.vector.memset(epst, EPS)
    onesd = consts.tile([P, 64], bf16, tag="onesd")
    nc.vector.memset(onesd, 1.0)

    # ---------- input DMAs (x first; weights gated) ----------
    x_sb = big.tile([P, NT, D], bf16, tag="x")
    nc.sync.dma_start(out=x_sb, in_=I["xc"])
    # packed [bq(4) | bk(4) | b1(16)] per-partition constants: one DMA
    bcons = consts.tile([P, 24], f32, tag="bcons")
    nc.sync.dma_start(out=bcons, in_=I["bcons"])
    bq_sb = bcons[:, 0:4]
    bk_sb = bcons[:, 4:8]
    b1_sb = bcons[:, 8:24]
    masks_sb = consts.tile([P, 1024], bf16, tag="masks")
    bo_sb = consts.tile([P, D], f32, tag="bo")
    b2_sb = consts.tile([P, D], f32, tag="b2")

    wq_sb = big.tile([P, ND, D], bf16, tag="wq")
    wk_sb = big.tile([P, ND, D], bf16, tag="wk")
    wv_sb = big.tile([P, ND, D], bf16, tag="wv")
    wo_sb = big.tile([P, ND, D], bf16, tag="wo")
    w1_sb = big.tile([P, ND, HIDDEN], fp8, tag="w1")
    w2_sb = big.tile([P, NHID, D], fp8, tag="w2")

    # ---------- LN1 (token-major stats; own tiles first) ----------
    xhat = big.tile([P, NT, D], bf16, tag="xhat")
    ln_order = [1, 2, 4, 5, 0, 3]
    for it, j in enumerate(ln_order):
        st = work.tile([P, 6], f32, tag="bnst")
        nc.vector.bn_stats(st, x_sb[:, j, :])
        # gates: release each weight DMA only once LN1 has consumed enough
        # of x, so x owns the HBM line first (real RAW dep, can't hoist).
        if it == 0:
            nc.vector.tensor_copy(wq_sb[0:1, 0:1, 0:1], st[0:1, 0:1])
        elif it == 2:
            nc.vector.tensor_copy(wk_sb[0:1, 0:1, 0:1], st[0:1, 0:1])
            nc.vector.tensor_copy(masks_sb[0:1, 0:1], st[0:1, 0:1])
        elif it == 4:
            nc.vector.tensor_copy(wv_sb[0:1, 0:1, 0:1], st[0:1, 0:1])
            nc.vector.tensor_copy(bo_sb[0:1, 0:1], st[0:1, 0:1])
            nc.vector.tensor_copy(b2_sb[0:1, 0:1], st[0:1, 0:1])
        mv = work.tile([P, 2], f32, tag="bnmv")
        nc.vector.bn_aggr(mv, st)
        r = work.tile([P, 1], f32, tag="lnr")
        nc.scalar.activation(r, mv[:, 1:2], AF.Sqrt, bias=epst, scale=1.0)
        r2 = work.tile([P, 1], f32, tag="lnr2")
        nc.vector.reciprocal(r2, r)
        nc.vector.tensor_scalar(
            out=xhat[:, j, :], in0=x_sb[:, j, :],
            scalar1=mv[:, 0:1], scalar2=r2,
            op0=OP.subtract, op1=OP.mult,
        )

    nc.sync.dma_start(out=wq_sb, in_=I["wqT"])
    nc.sync.dma_start(out=wk_sb, in_=I["wkT"])
    nc.sync.dma_start(out=wv_sb, in_=I["wvT"])
    nc.sync.dma_start(out=masks_sb, in_=I["masks"])
    nc.gpsimd.dma_start(out=bo_sb, in_=bcast(I["bo"]))
    nc.gpsimd.dma_start(out=b2_sb, in_=bcast(I["b2"]))

    # ---------- transpose x_hat -> x_hat^T [d, t] ----------
    xT = big.tile([P, ND, T], bf16, tag="xT")
    xTr = xT.rearrange("p d (s c) -> p d s c", s=2)   # [P, ND, 2, 384]
    for pi, j0 in enumerate((1, 4, 0)):
        j1 = j0 + 1 if j0 != 0 else 3
        for dt_ in range(ND):
            pt = pps.tile([P, 2 * P], bf16, tag="ps")
            for jj, j in enumerate((j0, j1)):
                nc.tensor.transpose(pt[:, jj * P:(jj + 1) * P],
                                    xhat[:, j, dt_ * P:(dt_ + 1) * P], ident)
            if j0 == 0:
                dst = xTr[:, dt_, :, 0:P]           # cols 0:128 and 384:512
            else:
                dst = xT[:, dt_, j0 * P:(j0 + 2) * P]
            if dt_ % 2 == 0:
                nc.vector.tensor_copy(dst, pt)
            else:
                nc.scalar.copy(dst, pt)

    # ---------- QKV ----------
    # Q^T [o, own t] (own tokens = stream cols 128:384 per stream)
    qT = big.tile([P, 4, CH], bf16, tag="qT")
    for ot in range(4):
        ps = pps.tile([P, CH], f32, tag="ps")
        for dt_ in range(ND):
            mm(ps, wq_sb[:, dt_, ot * P:(ot + 1) * P], xTr[:, dt_, :, P:SL],
               start=(dt_ == 0), stop=(dt_ == ND - 1))
        nc.scalar.activation(qT[:, ot, :], ps, AF.Identity,
                             bias=bq_sb[:, ot:ot + 1], scale=1.0)

    # K^T: own tokens first, then halo
    kT = big.tile([P, 4, T], bf16, tag="kT")
    kTr = kT.rearrange("p o (s c) -> p o s c", s=2)
    for ot in range(4):
        ps = pps.tile([P, CH], f32, tag="ps")
        for dt_ in range(ND):
            mm(ps, wk_sb[:, dt_, ot * P:(ot + 1) * P], xTr[:, dt_, :, P:SL],
               start=(dt_ == 0), stop=(dt_ == ND - 1))
        nc.scalar.activation(kTr[:, ot, :, P:SL], ps, AF.Identity,
                             bias=bk_sb[:, ot:ot + 1], scale=1.0)

    # V token-major per parity stream (own blocks first; halo last)
    v_sb = big.tile([P, 6, H, 65], bf16, tag="v")
    for i in range(6):
        nc.vector.memset(v_sb[:, i, :, 64:65], 1.0)

    def emit_v_block(stp, i):
        ps = pps.tile([P, D], f32, tag="ps")
        c0 = SL * stp + P * i
        for dt_ in range(ND):
            mm(ps, xT[:, dt_, c0:c0 + P], wv_sb[:, dt_, :],
               start=(dt_ == 0), stop=(dt_ == ND - 1))
        nc.vector.tensor_copy(
            v_sb[:, stp * 3 + i, :, 0:64],
            ps.rearrange("p (h c) -> p h c", h=H))

    for i in (1, 2):
        for stp in range(2):
            emit_v_block(stp, i)

    for ot in range(4):
        ps = pps.tile([P, 256], f32, tag="ps")
        for dt_ in range(ND):
            mm(ps, wk_sb[:, dt_, ot * P:(ot + 1) * P], xTr[:, dt_, :, 0:P],
               start=(dt_ == 0), stop=(dt_ == ND - 1))
        nc.scalar.activation(kTr[:, ot, :, 0:P], ps, AF.Identity,
                             bias=bk_sb[:, ot:ot + 1], scale=1.0)
        if ot == 1:
            # release late weights once x/wq/wk/wv traffic has drained
            nc.vector.tensor_copy(wo_sb[0:1, 0:1, 0:1], v_sb[0:1, 1, 0, 0:1])
            nc.vector.tensor_copy(w1_sb[0:1, 0:1, 0:1], v_sb[0:1, 1, 0, 0:1])
            nc.vector.tensor_copy(w2_sb[0:1, 0:1, 0:1], v_sb[0:1, 1, 0, 0:1])

    for stp in range(2):
        emit_v_block(stp, 0)

    nc.sync.dma_start(out=wo_sb, in_=I["woT"])
    nc.sync.dma_start(out=w1_sb, in_=I["w1T"])
    nc.sync.dma_start(out=w2_sb, in_=I["w2T"])

    # pre-add the out-proj bias into the residual source during slack time
    for tt in range(NO):
        nc.gpsimd.tensor_add(x_sb[:, OWN_TILE[tt], :],
                             x_sb[:, OWN_TILE[tt], :], bo_sb)

    # ---------- attention ----------
    # oU: staging [128, chain, q]; even chain ci: feat rows 0:64 + den row
    # 64; odd chain: den (replicated) rows 0:64 + feat rows 64:128.
    oU = big.tile([P, 8, CH], bf16, tag="oU")
    oT = big.tile([P, 4, CH], bf16, tag="oT")

    def emit_S(ci, hp, hh):
        lo = hh * 64
        sc = psc.tile([P, 1024], f32, tag="sc")
        for kt in range(3):
            qw = SQ if kt == 1 else P
            for stp in range(2):
                q0 = stp * SQ + (0 if kt < 2 else P)
                mm(sc[:, _scol(kt, stp):_scol(kt, stp) + qw],
                   kT[lo:lo + 64, hp, SL * stp + P * kt:SL * stp + P * kt + P],
                   qT[lo:lo + 64, hp, q0:q0 + qw],
                   start=True, stop=True)
        p_sb = pexp.tile([P, 1024], bf16, tag="p_sb")
        nc.scalar.activation(p_sb, sc, AF.Exp, scale=SCALE)
        nc.vector.tensor_mul(p_sb, p_sb, masks_sb)
        return p_sb

    def emit_PV(ci, hp, hh, p_sb):
        h = 2 * hp + hh
        po = pps.tile([P, CH], f32, tag="ps")
        ncol = 65 if hh == 0 else 64
        base = 0 if hh == 0 else 64
        for stp in range(2):
            qa = stp * SQ
            regions = (
                (qa, (_scol(0, stp), 0), (_scol(1, stp), 1)),
                (qa + P, (_scol(1, stp) + P, 1), (_scol(2, stp), 2)),
            )
            for q_out, (cA, iA), (cB, iB) in regions:
                mm(po[base:base + ncol, q_out:q_out + P],
                   v_sb[:, stp * 3 + iA, h, 0:ncol],
                   p_sb[:, cA:cA + P], start=True, stop=False)
                mm(po[base:base + ncol, q_out:q_out + P],
                   v_sb[:, stp * 3 + iB, h, 0:ncol],
                   p_sb[:, cB:cB + P], start=False, stop=True)
                if hh == 1:  # denominator, replicated into rows 0:64
                    mm(po[0:64, q_out:q_out + P], onesd,
                       p_sb[:, cA:cA + P], start=True, stop=False)
                    mm(po[0:64, q_out:q_out + P], onesd,
                       p_sb[:, cB:cB + P], start=False, stop=True)
        span = 65 if hh == 0 else P
        if ci in (1, 3):
            nc.scalar.copy(oU[0:span, ci, :], po[0:span, :])
        else:
            nc.vector.tensor_copy(oU[0:span, ci, :], po[0:span, :])

    def emit_norm(hp):
        rb_ps = pps.tile([P, CH], f32, tag="ps")
        mm(rb_ps[0:64, :], onesd[64:65, :], oU[64:65, 2 * hp, :],
           start=True, stop=True)
        mm(rb_ps[64:128, :], onesd[0:1, :], oU[0:1, 2 * hp + 1, :],
           start=True, stop=True)
        rb = work.tile([P, CH], f32, tag="rb")
        nc.vector.reciprocal_approx_fast(out=rb, in_=rb_ps)
        rbh = work.tile([P, CH], bf16, tag="rbh")
        nc.vector.tensor_copy(rbh, rb)
        for hh in range(2):
            lo = hh * 64
            nc.vector.tensor_mul(oT[lo:lo + 64, hp, :],
                                 oU[lo:lo + 64, 2 * hp + hh, :],
                                 rbh[lo:lo + 64, :])

    chains = [(hp, hh) for hp in range(4) for hh in range(2)]
    prev = None
    for ci, (hp, hh) in enumerate(chains):
        p_sb = emit_S(ci, hp, hh)
        if prev is not None:
            emit_PV(*prev)
            if prev[2] == 1:
                emit_norm(prev[1])
        prev = (ci, hp, hh, p_sb)
    emit_PV(*prev)
    emit_norm(prev[1])

    # ---------- out projection + residual ----------
    res1 = big.tile([P, NO, D], f32, tag="res1")
    for tt in range(NO):
        ps = pps.tile([P, D], f32, tag="ps")
        for dt_ in range(ND):
            mm(ps, oT[:, dt_, tt * P:(tt + 1) * P], wo_sb[:, dt_, :],
               start=(dt_ == 0), stop=(dt_ == ND - 1))
        nc.vector.tensor_add(res1[:, tt, :], ps, x_sb[:, OWN_TILE[tt], :])

    # ---------- LN2 ----------
    xhat2 = big.tile([P, NO, D], bf16, tag="xhat2")
    for j in range(NO):
        st = work.tile([P, 6], f32, tag="bnst")
        nc.vector.bn_stats(st, res1[:, j, :])
        mv = work.tile([P, 2], f32, tag="bnmv")
        nc.vector.bn_aggr(mv, st)
        r = work.tile([P, 1], f32, tag="lnr")
        nc.scalar.activation(r, mv[:, 1:2], AF.Sqrt, bias=epst, scale=1.0)
        r2 = work.tile([P, 1], f32, tag="lnr2")
        nc.vector.reciprocal(r2, r)
        nc.vector.tensor_scalar(
            out=xhat2[:, j, :], in0=res1[:, j, :],
            scalar1=mv[:, 0:1], scalar2=r2,
            op0=OP.subtract, op1=OP.mult,
        )

    for tt in range(NO):
        nc.gpsimd.tensor_add(res1[:, tt, :], res1[:, tt, :], b2_sb)

    x2T = big.tile([P, ND, CH], fp8, tag="x2T")
    for dt_ in range(ND):
        for j0 in (0, 2):
            pt = pps.tile([P, 2 * P], bf16, tag="ps")
            for jj in range(2):
                nc.tensor.transpose(pt[:, jj * P:(jj + 1) * P],
                                    xhat2[:, j0 + jj, dt_ * P:(dt_ + 1) * P], ident)
            if dt_ % 2 == 0:
                nc.vector.tensor_copy(x2T[:, dt_, j0 * P:(j0 + 2) * P], pt)
            else:
                nc.scalar.copy(x2T[:, dt_, j0 * P:(j0 + 2) * P], pt)

    # ---------- FFN, interleaved per ht-pair; fp8 DoubleRow ----------
    # FFN2 accumulates into 4 persistent psums (2 from the now-idle scores
    # pool, 2 from pps) while FFN1+gelu stream the hidden pairs through.
    g_sb = big.tile([P, NHID, CH], fp8, tag="g")
    fp0 = psc.tile([P, 1024], f32, tag="sc")
    fp1 = psc.tile([P, 1024], f32, tag="sc")
    fp2 = pps.tile([P, D], f32, tag="ps")
    fp3 = pps.tile([P, D], f32, tag="ps")
    fview = [fp0[:, 0:D], fp1[:, 0:D], fp2, fp3]
    def emit_ffn2(pr):
        for tt in range(NO):
            mm(fview[tt], g_sb[:, 2 * pr:2 * pr + 2, tt * P:(tt + 1) * P],
               w2_sb[:, 2 * pr:2 * pr + 2, :],
               start=(pr == 0), stop=(pr == NHID // 2 - 1),
               perf_mode=PM.DoubleRow)

    for pr in range(NHID // 2):
        for ht in (2 * pr, 2 * pr + 1):
            ps = pps.tile([P, CH], f32, tag="ps")
            for dt_ in (0, 2):
                mm(ps, w1_sb[:, dt_:dt_ + 2, ht * P:(ht + 1) * P],
                   x2T[:, dt_:dt_ + 2, :],
                   start=(dt_ == 0), stop=(dt_ == 2), perf_mode=PM.DoubleRow)
            nc.scalar.activation(g_sb[:, ht, :], ps, AF.Gelu,
                                 bias=b1_sb[:, ht:ht + 1],
                                 scale=1.0 / FFN_WSCALE)
        if pr > 0:
            emit_ffn2(pr - 1)   # one pair behind: gelu latency stays hidden
    emit_ffn2(NHID // 2 - 1)

    fin = big.tile([P, NO, D], bf16, tag="fin")
    yr = y.rearrange("(j p) d -> p j d", p=P)
    for tt in range(NO):
        nc.vector.scalar_tensor_tensor(
            out=fin[:, tt, :], in0=fview[tt], scalar=1.0 / FFN_WSCALE,
            in1=res1[:, tt, :], op0=OP.mult, op1=OP.add)
        nc.sync.dma_start(out=yr[:, tt, :], in_=fin[:, tt, :])


def _build():
    from contextlib import ExitStack

    import concourse.bacc as bacc
    import concourse.tile as tile
    from concourse import mybir

    f32 = mybir.dt.float32
    bf16 = mybir.dt.bfloat16
    fp8 = mybir.dt.float8e4
    nc = bacc.Bacc("TRN2", target_bir_lowering=False, debug=False,
                   enable_asserts=False, num_devices=NCORES)
    I = {}

    def inp(name, shape, dt_):
        I[name] = nc.dram_tensor(name, list(shape), dt_, kind="ExternalInput").ap()

    inp("xc", (P, NT, D), bf16)
    inp("wqT", (P, ND, D), bf16)
    inp("wkT", (P, ND, D), bf16)
    inp("wvT", (P, ND, D), bf16)
    inp("bcons", (P, 24), f32)
    inp("woT", (P, ND, D), bf16)
    inp("bo", (D,), f32)
    inp("w1T", (P, ND, HIDDEN), fp8)
    inp("w2T", (P, NHID, D), fp8)
    inp("b2", (D,), f32)
    inp("masks", (P, 1024), bf16)
    y = nc.dram_tensor("y", [CH, D], bf16, kind="ExternalOutput").ap()

    with tile.TileContext(nc) as tc:
        with ExitStack() as ctx:
            _body(ctx, tc, I, y)
    nc.compile()
    return nc


def _host_masks():
    import ml_dtypes
    sk = np.arange(SL)[:, None]
    sq = np.arange(SL - SQ, SL)[None, :]
    valid = ((sq - sk >= 0) & (sq - sk <= SW)).astype(np.float32)  # [384, 256]
    kt0 = valid[0:P, 0:P]           # keys 0:128, queries 0:128
    kt1 = valid[P:2 * P, :]         # keys 128:256, all queries
    kt2 = valid[2 * P:3 * P, P:SQ]  # keys 256:384, queries 128:256
    m = np.concatenate([kt0, kt0, kt1, kt1, kt2, kt2], axis=1)  # [128, 1024]
    m = m.astype(ml_dtypes.bfloat16)
    m0 = m.copy()
    m0[:, 0:256] = 0.0  # first chunk of each batch: halo keys invalid
    return np.ascontiguousarray(m), np.ascontiguousarray(m0)


def get_nc():
    global _nc
    if _nc is None:
        _nc = _build()
    return _nc


def _pmaj(a, p=P):
    """[N*p, F...] row-major -> [p, N, F...] partition-major contiguous."""
    n = a.shape[0] // p
    return np.ascontiguousarray(
        a.reshape((n, p) + a.shape[1:]).transpose((1, 0) + tuple(range(2, a.ndim + 1))))


def make_in_maps(inputs):
    import ml_dtypes
    f = np.float32
    bf = ml_dtypes.bfloat16
    x = np.asarray(inputs["x"], f)
    qkv_w = np.asarray(inputs["qkv_w"], f)
    n1w = np.asarray(inputs["norm1_w"], f)
    n1b = np.asarray(inputs["norm1_b"], f)
    wqkv_f = qkv_w * n1w[None, :]
    bqkv = qkv_w @ n1b + np.asarray(inputs["qkv_b"], f)
    wT = np.ascontiguousarray(wqkv_f.T)        # [D, 3D]
    wqT = _pmaj(wT[:, 0:D].copy().astype(bf))
    wkT = _pmaj(wT[:, D:2 * D].copy().astype(bf))
    wvT = _pmaj(wT[:, 2 * D:3 * D].copy().astype(bf))
    bq = np.ascontiguousarray(bqkv[0:D].reshape(4, P).T)
    bk = np.ascontiguousarray(bqkv[D:2 * D].reshape(4, P).T)
    bv = bqkv[2 * D:3 * D]

    out_w = np.asarray(inputs["out_w"], f)
    woT = _pmaj(np.ascontiguousarray(out_w.T).astype(bf))
    # fold the V bias through the out projection (softmax weights sum to 1)
    bo = np.ascontiguousarray(np.asarray(inputs["out_b"], f) + out_w @ bv)

    fp8 = ml_dtypes.float8_e4m3
    w1 = np.asarray(inputs["ffn_w1"], f)
    n2w = np.asarray(inputs["norm2_w"], f)
    n2b = np.asarray(inputs["norm2_b"], f)
    w1T = _pmaj(np.clip(np.ascontiguousarray((w1 * n2w[None, :]).T)
                        * FFN_WSCALE, -240, 240).astype(fp8))
    b1v = w1 @ n2b + np.asarray(inputs["ffn_b1"], f)
    b1 = np.ascontiguousarray(b1v.reshape(NHID, P).T)
    w2T = _pmaj(np.clip(np.ascontiguousarray(np.asarray(inputs["ffn_w2"], f).T)
                        * FFN_WSCALE, -240, 240).astype(fp8))
    b2 = np.ascontiguousarray(np.asarray(inputs["ffn_b2"], f))

    bcons = np.ascontiguousarray(np.concatenate([bq, bk, b1], axis=1))
    masks, masks0 = _host_masks()
    shared = dict(wqT=wqT, wkT=wkT, wvT=wvT, bcons=bcons, woT=woT, bo=bo,
                  w1T=w1T, w2T=w2T, b2=b2)
    # stream-major permutation of the 768 halo+own tokens
    perm = np.concatenate([np.arange(0, T, 2), np.arange(1, T, 2)])
    in_maps = []
    for c in range(NCORES):
        b_, i = divmod(c, 4)
        own = x[b_, i * CH:(i + 1) * CH]
        if i == 0:
            halo = np.zeros((HALO, D), f)
        else:
            halo = x[b_, i * CH - HALO:i * CH]
        xc = np.concatenate([halo, own], 0)[perm]
        xc = _pmaj(xc.astype(bf))
        in_maps.append(dict(xc=xc, masks=(masks if i > 0 else masks0), **shared))
    return in_maps


def kernel(**inputs):
    global LAST_EXEC_NS, LAST_RESULTS
    from concourse.bass_utils import run_bass_kernel_spmd

    nc = get_nc()
    in_maps = make_in_maps(inputs)
    trace = bool(int(os.environ.get("BASS_KERNEL_TRACE", "0")))
    res = run_bass_kernel_spmd(nc, in_maps, core_ids=list(range(NCORES)),
                               trace=trace)
    LAST_EXEC_NS = res.exec_time_ns
    LAST_RESULTS = res
    out = np.zeros((B, L, D), np.float32)
    # kernel y rows are stream-major own tokens: un-permute
    operm = np.concatenate([np.arange(0, CH, 2), np.arange(1, CH, 2)])
    for c, r in enumerate(res.results):
        b_, i = divmod(c, 4)
        out[b_, i * CH + operm] = np.asarray(r["y"], np.float32)
    return out
